# revision 1
# baseline (speedup 1.0000x reference)
"""Trainium2 Bass kernel for nn_DegradationModel (dense_mlp).

Math: the MLPs use ReLU activations, so each scalar network y(c, o, f) is
piecewise-linear in its post-transform input x = [c*s(f0), o, f1:], with
s(f0) = 1e-10 + exp(-f0) the only nonlinearity. Hence all second derivatives
vanish except through x0 = c*s(f0):
    d2C = 0,  d2O = 0,  d2F[i,j] = 0 except d2F[0,0] = g0 * c * exp(-f0)
where g0 = dy/dx0. Also dC = g0 * s(f0), so
    predicted = y + g0 * s(f0) * (measured - c).
The kernel therefore only needs a forward pass plus a backward pass for the
single gradient component g0 per row. The huge [N*V, 64, 64] Hessian outputs
are zero except the [0,0] element of each row block; the SPMD runner seeds
ExternalOutput buffers with donated zero buffers, so only the nonzero
elements are written.

Sharding: pure data parallel over the N=512 center rows, 64 rows per core
across 8 cores; weights/dictionary replicated.
"""

import sys

if "/opt/trn_rl_repo" not in sys.path:
    sys.path.insert(0, "/opt/trn_rl_repo")

import numpy as np

W = 64          # feature width
V = 32          # voltages
N = 512         # total rows
NCORES = 8
NSH = N // NCORES      # 64 rows per core
MROW = NSH * V         # 2048 cap rows per core
DEPTH = 3
CAP_IN = W + 3  # 67
VOL_IN = W + 2  # 66
KAUG = NSH + V  # 96: contraction dim for the broadcast matmul
K_DICT = 256
CH = 512        # matmul free-dim chunk (one PSUM bank)

_CACHE = {}


def _build_program():
    import concourse.bass as bass
    import concourse.bacc as bacc
    import concourse.mybir as mybir
    from concourse.tile import TileContext
    from concourse.masks import make_identity

    dt = mybir.dt
    f32 = dt.float32
    Alu = mybir.AluOpType
    Act = mybir.ActivationFunctionType

    nc = bacc.Bacc("TRN2", target_bir_lowering=False, debug=False,
                   num_devices=NCORES)

    # ---- I/O ----
    d_centers = nc.dram_tensor("centers", [NSH, 3], f32, kind="ExternalInput")
    d_meas = nc.dram_tensor("meas", [NSH, 1], f32, kind="ExternalInput")
    d_volt = nc.dram_tensor("volt", [V, 1], f32, kind="ExternalInput")
    d_ind = nc.dram_tensor("ind", [NSH, 1], dt.int32, kind="ExternalInput")
    d_dict = nc.dram_tensor("dictk", [K_DICT, 2 * W], f32, kind="ExternalInput")
    d_eps = nc.dram_tensor("epsk", [K_DICT, W], f32, kind="ExternalInput")
    d_cw0 = nc.dram_tensor("cw0", [CAP_IN, W], f32, kind="ExternalInput")
    d_cb0 = nc.dram_tensor("cb0", [1, W], f32, kind="ExternalInput")
    d_cwb = nc.dram_tensor("cwb", [DEPTH, W, W], f32, kind="ExternalInput")
    d_cbb = nc.dram_tensor("cbb", [DEPTH, W], f32, kind="ExternalInput")
    d_cwf = nc.dram_tensor("cwf", [W, 1], f32, kind="ExternalInput")
    d_cbf = nc.dram_tensor("cbf", [1, 1], f32, kind="ExternalInput")
    d_vw0 = nc.dram_tensor("vw0", [VOL_IN, W], f32, kind="ExternalInput")
    d_vb0 = nc.dram_tensor("vb0", [1, W], f32, kind="ExternalInput")
    d_vwb = nc.dram_tensor("vwb", [DEPTH, W, W], f32, kind="ExternalInput")
    d_vbb = nc.dram_tensor("vbb", [DEPTH, W], f32, kind="ExternalInput")
    d_vwf = nc.dram_tensor("vwf", [W, 1], f32, kind="ExternalInput")
    d_vbf = nc.dram_tensor("vbf", [1, 1], f32, kind="ExternalInput")

    d_pcap = nc.dram_tensor("pcap", [NSH, V], f32, kind="ExternalOutput")
    d_pvol = nc.dram_tensor("pvol", [NSH, 1], f32, kind="ExternalOutput")
    d_mean = nc.dram_tensor("meano", [NSH, W], f32, kind="ExternalOutput")
    d_lsig = nc.dram_tensor("lsigo", [NSH, W], f32, kind="ExternalOutput")
    d_cd2f = nc.dram_tensor("cd2f", [MROW, W * W], f32, kind="ExternalOutput")
    d_vd2f = nc.dram_tensor("vd2f", [NSH, W * W], f32, kind="ExternalOutput")

    with TileContext(nc) as tc:
        with (
            tc.tile_pool(name="const", bufs=1) as cp,
            tc.tile_pool(name="work", bufs=2) as wp,
            tc.tile_pool(name="vbuf", bufs=3) as vp,
            tc.tile_pool(name="psum", bufs=4, space="PSUM") as pp,
            tc.tile_pool(name="psmall", bufs=3, space="PSUM") as sp,
        ):
            # ---------- gathers + features ----------
            ind_t = cp.tile([NSH, 1], dt.int32)
            nc.sync.dma_start(out=ind_t[:], in_=d_ind[:, :])
            gath = cp.tile([NSH, 2 * W], f32)
            nc.gpsimd.indirect_dma_start(
                out=gath[:], out_offset=None, in_=d_dict[:, :],
                in_offset=bass.IndirectOffsetOnAxis(ap=ind_t[:, :1], axis=0))
            epsg = cp.tile([NSH, W], f32)
            nc.gpsimd.indirect_dma_start(
                out=epsg[:], out_offset=None, in_=d_eps[:, :],
                in_offset=bass.IndirectOffsetOnAxis(ap=ind_t[:, :1], axis=0))
            nc.sync.dma_start(out=d_mean[:, :], in_=gath[:, 0:W])
            nc.sync.dma_start(out=d_lsig[:, :], in_=gath[:, W:2 * W])

            sig = cp.tile([NSH, W], f32)
            nc.scalar.activation(out=sig[:], in_=gath[:, W:2 * W],
                                 func=Act.Exp, scale=0.5)
            feat = cp.tile([NSH, W], f32)
            nc.vector.tensor_tensor(out=feat[:], in0=sig[:], in1=epsg[:],
                                    op=Alu.mult)
            nc.vector.tensor_tensor(out=feat[:], in0=feat[:], in1=gath[:, 0:W],
                                    op=Alu.add)

            # ---------- per-row scalars ----------
            cvec = cp.tile([NSH, 1], f32)
            nc.sync.dma_start(out=cvec[:], in_=d_centers[:, 0:1])
            o0 = cp.tile([NSH, 1], f32)
            nc.sync.dma_start(out=o0[:], in_=d_centers[:, 1:2])
            o1 = cp.tile([NSH, 1], f32)
            nc.sync.dma_start(out=o1[:], in_=d_centers[:, 2:3])
            mc = cp.tile([NSH, 1], f32)
            nc.sync.dma_start(out=mc[:], in_=d_meas[:, :])
            voltc = cp.tile([V, 1], f32)
            nc.sync.dma_start(out=voltc[:], in_=d_volt[:, :])

            evec = cp.tile([NSH, 1], f32)
            nc.scalar.activation(out=evec[:], in_=feat[:, 0:1],
                                 func=Act.Exp, scale=-1.0)
            svec = cp.tile([NSH, 1], f32)
            nc.vector.tensor_scalar_add(out=svec[:], in0=evec[:], scalar1=1e-10)
            csv = cp.tile([NSH, 1], f32)
            nc.vector.tensor_tensor(out=csv[:], in0=cvec[:], in1=svec[:],
                                    op=Alu.mult)
            cev = cp.tile([NSH, 1], f32)
            nc.vector.tensor_tensor(out=cev[:], in0=cvec[:], in1=evec[:],
                                    op=Alu.mult)
            varv = cp.tile([NSH, 1], f32)
            nc.vector.tensor_tensor(out=varv[:], in0=mc[:], in1=cvec[:],
                                    op=Alu.subtract)
            svv = cp.tile([NSH, 1], f32)
            nc.vector.tensor_tensor(out=svv[:], in0=svec[:], in1=varv[:],
                                    op=Alu.mult)

            # ---------- augmented per-row input block XsAug [96, 67] ----------
            # rows 0:64 -> per-n input [cs, o0, o1, 0, f1..f63]
            # rows 64:96 -> voltage rows: only col 3 = volt[v]
            xsa = cp.tile([KAUG, CAP_IN], f32)
            nc.gpsimd.memset(xsa[:], 0.0)
            nc.vector.tensor_copy(out=xsa[0:NSH, 0:1], in_=csv[:])
            nc.vector.tensor_copy(out=xsa[0:NSH, 1:2], in_=o0[:])
            nc.vector.tensor_copy(out=xsa[0:NSH, 2:3], in_=o1[:])
            nc.vector.tensor_copy(out=xsa[0:NSH, 4:CAP_IN], in_=feat[:, 1:W])
            nc.vector.tensor_copy(out=xsa[NSH:KAUG, 3:4], in_=voltc[:])

            # ---------- broadcast/selection matrix R [96, 2048] ----------
            # R[n, j] = (j // V == n); R[64+v, j] = (j % V == v)
            # so XsAug.T @ R = full cap input (features on partitions,
            # batch n-major over the free dim) including the voltage row.
            ri = cp.tile([KAUG, MROW], dt.int32)
            ci = cp.tile([KAUG, MROW], dt.int32)
            Rm = cp.tile([KAUG, MROW], f32)
            nc.gpsimd.iota(ri[:], pattern=[[0, MROW]], base=0,
                           channel_multiplier=1)
            nc.gpsimd.iota(ci[0:NSH, :], pattern=[[1, NSH], [0, V]], base=0,
                           channel_multiplier=0)
            nc.gpsimd.iota(ci[NSH:KAUG, :], pattern=[[0, NSH], [1, V]],
                           base=NSH, channel_multiplier=0)
            nc.vector.tensor_tensor(out=Rm[:], in0=ri[:], in1=ci[:],
                                    op=Alu.is_equal)

            # ---------- weights/biases to SBUF ----------
            ident = cp.tile([W, W], f32)
            make_identity(nc, ident[:])

            def load_net(dw0, db0, dwb, dbb, dwf, dbf, in_dim, pfx):
                w0s = cp.tile([in_dim, W], f32, tag=pfx + "w0")
                nc.sync.dma_start(out=w0s[:], in_=dw0[:, :])
                w0c = cp.tile([W, 1], f32, tag=pfx + "w0c")
                nc.sync.dma_start(out=w0c[:], in_=dw0[0:1, :])
                b0c = cp.tile([W, 1], f32, tag=pfx + "b0c")
                nc.sync.dma_start(out=b0c[:], in_=db0[0:1, :])
                wfs = cp.tile([W, 1], f32, tag=pfx + "wf")
                nc.sync.dma_start(out=wfs[:], in_=dwf[:, :])
                bfc = cp.tile([1, 1], f32, tag=pfx + "bf")
                nc.sync.dma_start(out=bfc[:], in_=dbf[:, :])
                wbs, wbT, bbc = [], [], []
                for i in range(DEPTH):
                    wb_i = cp.tile([W, W], f32, tag=f"{pfx}wb{i}")
                    nc.sync.dma_start(out=wb_i[:], in_=dwb[i, :, :])
                    wbs.append(wb_i)
                    pst = sp.tile([W, W], f32, tag="sm")
                    nc.tensor.transpose(out=pst[:], in_=wb_i[:],
                                        identity=ident[:])
                    wt_i = cp.tile([W, W], f32, tag=f"{pfx}wbT{i}")
                    nc.vector.tensor_copy(out=wt_i[:], in_=pst[:])
                    wbT.append(wt_i)
                    bb_i = cp.tile([W, 1], f32, tag=f"{pfx}bb{i}")
                    nc.sync.dma_start(out=bb_i[:], in_=dbb[i:i + 1, :])
                    bbc.append(bb_i)
                return w0s, w0c, b0c, wbs, wbT, bbc, wfs, bfc

            cw0s, cw0c, cb0c, cwbs, cwbT, cbbc, cwfs, cbfc = load_net(
                d_cw0, d_cb0, d_cwb, d_cbb, d_cwf, d_cbf, CAP_IN, "c")
            vw0s, vw0c, vb0c, vwbs, vwbT, vbbc, vwfs, vbfc = load_net(
                d_vw0, d_vb0, d_vwb, d_vbb, d_vwf, d_vbf, VOL_IN, "v")

            # ---------- cap net: forward + g0 backward, 4 chunks ----------
            yrow = cp.tile([1, MROW], f32)
            grow = cp.tile([1, MROW], f32)
            for k in range(MROW // CH):
                sl = slice(k * CH, (k + 1) * CH)
                px = pp.tile([CAP_IN, CH], f32, tag="mm")
                nc.tensor.matmul(out=px[:], lhsT=xsa[:], rhs=Rm[:, sl],
                                 start=True, stop=True)
                xt = wp.tile([CAP_IN, CH], f32, tag="xt")
                nc.scalar.copy(out=xt[:], in_=px[:])

                p0 = pp.tile([W, CH], f32, tag="mm")
                nc.tensor.matmul(out=p0[:], lhsT=cw0s[:], rhs=xt[:],
                                 start=True, stop=True)
                h = []
                h0 = wp.tile([W, CH], f32, tag="h0")
                nc.scalar.activation(out=h0[:], in_=p0[:], func=Act.Relu,
                                     bias=cb0c[:], scale=1.0)
                h.append(h0)
                for i in range(DEPTH):
                    pi = pp.tile([W, CH], f32, tag="mm")
                    nc.tensor.matmul(out=pi[:], lhsT=cwbs[i][:], rhs=h[-1][:],
                                     start=True, stop=True)
                    hi = wp.tile([W, CH], f32, tag=f"h{i + 1}")
                    nc.scalar.activation(out=hi[:], in_=pi[:], func=Act.Relu,
                                         bias=cbbc[i][:], scale=1.0)
                    h.append(hi)
                py = pp.tile([1, CH], f32, tag="mm")
                nc.tensor.matmul(out=py[:], lhsT=cwfs[:], rhs=h[DEPTH][:],
                                 start=True, stop=True)
                nc.vector.tensor_scalar(out=yrow[:, sl], in0=py[:],
                                        scalar1=cbfc[:, 0:1], scalar2=None,
                                        op0=Alu.add)
                # backward for g0 = dy/dx0
                v = vp.tile([W, CH], f32, tag="v")
                nc.vector.tensor_scalar(out=v[:], in0=h[DEPTH][:], scalar1=0.0,
                                        scalar2=cwfs[:, 0:1], op0=Alu.is_gt,
                                        op1=Alu.mult)
                for i in range(DEPTH - 1, -1, -1):
                    pb = pp.tile([W, CH], f32, tag="mm")
                    nc.tensor.matmul(out=pb[:], lhsT=cwbT[i][:], rhs=v[:],
                                     start=True, stop=True)
                    v2 = vp.tile([W, CH], f32, tag="v")
                    nc.vector.scalar_tensor_tensor(
                        out=v2[:], in0=h[i][:], scalar=0.0, in1=pb[:],
                        op0=Alu.is_gt, op1=Alu.mult)
                    v = v2
                pg = pp.tile([1, CH], f32, tag="mm")
                nc.tensor.matmul(out=pg[:], lhsT=cw0c[:], rhs=v[:],
                                 start=True, stop=True)
                nc.vector.tensor_copy(out=grow[:, sl], in_=pg[:])

            # ---------- vol net (single 64-wide batch) ----------
            xv = cp.tile([NSH, VOL_IN], f32)
            nc.gpsimd.memset(xv[:], 0.0)
            nc.vector.tensor_copy(out=xv[:, 0:1], in_=csv[:])
            nc.vector.tensor_copy(out=xv[:, 1:2], in_=o0[:])
            nc.vector.tensor_copy(out=xv[:, 2:3], in_=o1[:])
            nc.vector.tensor_copy(out=xv[:, 3:VOL_IN], in_=feat[:, 1:W])
            pxv = sp.tile([VOL_IN, NSH], f32, tag="sm")
            nc.tensor.transpose(out=pxv[:], in_=xv[:], identity=ident[:])
            xvt = cp.tile([VOL_IN, NSH], f32)
            nc.vector.tensor_copy(out=xvt[:], in_=pxv[:])

            p0v = sp.tile([W, NSH], f32, tag="sm")
            nc.tensor.matmul(out=p0v[:], lhsT=vw0s[:], rhs=xvt[:],
                             start=True, stop=True)
            hv = []
            hv0 = cp.tile([W, NSH], f32, tag="hv0")
            nc.scalar.activation(out=hv0[:], in_=p0v[:], func=Act.Relu,
                                 bias=vb0c[:], scale=1.0)
            hv.append(hv0)
            for i in range(DEPTH):
                piv = sp.tile([W, NSH], f32, tag="sm")
                nc.tensor.matmul(out=piv[:], lhsT=vwbs[i][:], rhs=hv[-1][:],
                                 start=True, stop=True)
                hvi = cp.tile([W, NSH], f32, tag=f"hv{i + 1}")
                nc.scalar.activation(out=hvi[:], in_=piv[:], func=Act.Relu,
                                     bias=vbbc[i][:], scale=1.0)
                hv.append(hvi)
            pyv = sp.tile([1, NSH], f32, tag="sm")
            nc.tensor.matmul(out=pyv[:], lhsT=vwfs[:], rhs=hv[DEPTH][:],
                             start=True, stop=True)
            yvrow = cp.tile([1, NSH], f32)
            nc.vector.tensor_scalar(out=yvrow[:], in0=pyv[:],
                                    scalar1=vbfc[:, 0:1], scalar2=None,
                                    op0=Alu.add)
            uv = cp.tile([W, NSH], f32, tag="uv3")
            nc.vector.tensor_scalar(out=uv[:], in0=hv[DEPTH][:], scalar1=0.0,
                                    scalar2=vwfs[:, 0:1], op0=Alu.is_gt,
                                    op1=Alu.mult)
            for i in range(DEPTH - 1, -1, -1):
                pbv = sp.tile([W, NSH], f32, tag="sm")
                nc.tensor.matmul(out=pbv[:], lhsT=vwbT[i][:], rhs=uv[:],
                                 start=True, stop=True)
                uv2 = cp.tile([W, NSH], f32, tag=f"uv{i}")
                nc.vector.scalar_tensor_tensor(
                    out=uv2[:], in0=hv[i][:], scalar=0.0, in1=pbv[:],
                    op0=Alu.is_gt, op1=Alu.mult)
                uv = uv2
            pgv = sp.tile([1, NSH], f32, tag="sm")
            nc.tensor.matmul(out=pgv[:], lhsT=vw0c[:], rhs=uv[:],
                             start=True, stop=True)
            gvrow = cp.tile([1, NSH], f32)
            nc.vector.tensor_copy(out=gvrow[:], in_=pgv[:])

            # ---------- epilogue ----------
            y2 = cp.tile([NSH, V], f32)
            nc.sync.dma_start(out=y2[:], in_=yrow[:, :])
            g2 = cp.tile([NSH, V], f32)
            nc.sync.dma_start(out=g2[:], in_=grow[:, :])
            pc_t = cp.tile([NSH, V], f32)
            nc.vector.scalar_tensor_tensor(out=pc_t[:], in0=g2[:],
                                           scalar=svv[:, 0:1], in1=y2[:],
                                           op0=Alu.mult, op1=Alu.add)
            d2_t = cp.tile([NSH, V], f32)
            nc.vector.tensor_scalar_mul(out=d2_t[:], in0=g2[:],
                                        scalar1=cev[:, 0:1])
            nc.sync.dma_start(out=d_pcap[:, :], in_=pc_t[:])
            nc.sync.dma_start(out=d_cd2f[:, 0:1], in_=d2_t[:])

            yv2 = cp.tile([NSH, 1], f32)
            nc.sync.dma_start(out=yv2[:], in_=yvrow[:, :])
            gv2 = cp.tile([NSH, 1], f32)
            nc.sync.dma_start(out=gv2[:], in_=gvrow[:, :])
            pv_t = cp.tile([NSH, 1], f32)
            nc.vector.scalar_tensor_tensor(out=pv_t[:], in0=gv2[:],
                                           scalar=svv[:, 0:1], in1=yv2[:],
                                           op0=Alu.mult, op1=Alu.add)
            d2v_t = cp.tile([NSH, 1], f32)
            nc.vector.tensor_tensor(out=d2v_t[:], in0=gv2[:], in1=cev[:],
                                    op=Alu.mult)
            nc.sync.dma_start(out=d_pvol[:, :], in_=pv_t[:])
            nc.sync.dma_start(out=d_vd2f[:, 0:1], in_=d2v_t[:])

    nc.compile()
    return nc


def _get_program():
    if "nc" not in _CACHE:
        _CACHE["nc"] = _build_program()
    return _CACHE["nc"]


def kernel(**inputs):
    from concourse.bass_utils import run_bass_kernel_spmd

    inp = {k: np.ascontiguousarray(np.asarray(v)) for k, v in inputs.items()}
    nc = _get_program()

    f32 = np.float32
    shared = {
        "volt": inp["voltages"].reshape(V, 1).astype(f32),
        "dictk": inp["dict_kernel"].astype(f32),
        "epsk": inp["eps"].astype(f32),
        "cw0": inp["cap_w0"].astype(f32),
        "cb0": inp["cap_b0"].reshape(1, W).astype(f32),
        "cwb": inp["cap_wb"].astype(f32),
        "cbb": inp["cap_bb"].astype(f32),
        "cwf": inp["cap_wf"].reshape(W, 1).astype(f32),
        "cbf": inp["cap_bf"].reshape(1, 1).astype(f32),
        "vw0": inp["vol_w0"].astype(f32),
        "vb0": inp["vol_b0"].reshape(1, W).astype(f32),
        "vwb": inp["vol_wb"].astype(f32),
        "vbb": inp["vol_bb"].astype(f32),
        "vwf": inp["vol_wf"].reshape(W, 1).astype(f32),
        "vbf": inp["vol_bf"].reshape(1, 1).astype(f32),
    }
    ind32 = inp["indecies"].astype(np.int32).reshape(N, 1)
    centers = inp["centers"].astype(f32)
    meas = inp["measured_cycles"].astype(f32).reshape(N, 1)

    in_maps = []
    for c in range(NCORES):
        sl = slice(c * NSH, (c + 1) * NSH)
        m = dict(shared)
        m["centers"] = np.ascontiguousarray(centers[sl])
        m["meas"] = np.ascontiguousarray(meas[sl])
        m["ind"] = np.ascontiguousarray(ind32[sl])
        in_maps.append(m)

    res = run_bass_kernel_spmd(nc, in_maps, core_ids=list(range(NCORES)),
                               trace=False).results

    predicted_cap = np.concatenate([r["pcap"] for r in res], axis=0)
    predicted_vol = np.concatenate([r["pvol"][:, 0] for r in res], axis=0)
    mean = np.concatenate([r["meano"] for r in res], axis=0)
    log_sig = np.concatenate([r["lsigo"] for r in res], axis=0)
    cd2F = np.concatenate([r["cd2f"] for r in res], axis=0).reshape(N * V, W, W)
    vd2F = np.concatenate([r["vd2f"] for r in res], axis=0).reshape(N, W, W)
    return predicted_cap, predicted_vol, mean, log_sig, cd2F, vd2F


# revision 2
# speedup vs baseline: 1.5718x; 1.5718x over previous
"""Trainium2 Bass kernel for nn_DegradationModel (dense_mlp).

Math: the MLPs use ReLU activations, so each scalar network y(c, o, f) is
piecewise-linear in its post-transform input x = [c*s(f0), o, f1:], with
s(f0) = 1e-10 + exp(-f0) the only nonlinearity. Hence all second derivatives
vanish except through x0 = c*s(f0):
    d2C = 0,  d2O = 0,  d2F[i,j] = 0 except d2F[0,0] = g0 * c * exp(-f0)
where g0 = dy/dx0. Also dC = g0 * s(f0), so
    predicted = y + g0 * s(f0) * (measured - c).
The kernel therefore only needs a forward pass plus a backward pass for the
single gradient component g0 per row. The huge [N*V, 64, 64] Hessian outputs
are zero except the [0,0] element of each row block; the SPMD runner seeds
ExternalOutput buffers with donated zero buffers, so only the nonzero
elements are written.

Precision: the forward hidden layers run in fp32 (ReLU masks are
discontinuous, so pre-activations must match the fp32 reference closely);
the backward pass is linear once the masks are fixed, so those matmuls (and
the final y readout) run in fp16 (measured end-to-end error ~1e-3).

Layer 0 exploits structure: cap rows (n, v) share everything except the
voltage, so with B = [Xs; Volt] @ W0 (computed once on the PE), the layer-0
pre-activation for column (n, v) is B.T[:, n] + B.T[:, 64+v] — a broadcast
add on the vector engine instead of 2048-column matmuls.

Sharding: pure data parallel over the N=512 center rows, 64 rows per core
across 8 cores; weights/dictionary replicated.
"""

import sys

if "/opt/trn_rl_repo" not in sys.path:
    sys.path.insert(0, "/opt/trn_rl_repo")

import numpy as np

W = 64          # feature width
V = 32          # voltages
N = 512         # total rows
NCORES = 8
NSH = N // NCORES      # 64 rows per core
MROW = NSH * V         # 2048 cap rows per core
DEPTH = 3
CAP_IN = W + 3  # 67
VOL_IN = W + 2  # 66
KAUG = NSH + V  # 96
K_DICT = 256
CH = 512        # matmul free-dim chunk (one PSUM bank)
NCH = MROW // CH       # 4 chunks
NB = CH // V           # 16 n-rows per chunk

_CACHE = {}


def _build_program():
    import concourse.bass as bass
    import concourse.bacc as bacc
    import concourse.mybir as mybir
    from concourse.tile import TileContext

    dt = mybir.dt
    f32 = dt.float32
    f16 = dt.float16
    Alu = mybir.AluOpType
    Act = mybir.ActivationFunctionType

    nc = bacc.Bacc("TRN2", target_bir_lowering=False, debug=False,
                   num_devices=NCORES)

    # ---- I/O ----
    d_centers = nc.dram_tensor("centers", [NSH, 3], f32, kind="ExternalInput")
    d_meas = nc.dram_tensor("meas", [NSH, 1], f32, kind="ExternalInput")
    d_volt = nc.dram_tensor("volt", [V, 1], f32, kind="ExternalInput")
    d_ind = nc.dram_tensor("ind", [NSH, 1], dt.int32, kind="ExternalInput")
    d_dict = nc.dram_tensor("dictk", [K_DICT, 2 * W], f32, kind="ExternalInput")
    d_eps = nc.dram_tensor("epsk", [K_DICT, W], f32, kind="ExternalInput")
    d_ident = nc.dram_tensor("ident96", [KAUG, KAUG], f32, kind="ExternalInput")
    d_cw0 = nc.dram_tensor("cw0", [CAP_IN, W], f32, kind="ExternalInput")
    d_cb0 = nc.dram_tensor("cb0", [1, W], f32, kind="ExternalInput")
    d_cwb = nc.dram_tensor("cwb", [DEPTH, W, W], f32, kind="ExternalInput")
    d_cbb = nc.dram_tensor("cbb", [DEPTH, W], f32, kind="ExternalInput")
    d_cwf = nc.dram_tensor("cwf", [W, 1], f32, kind="ExternalInput")
    d_cbf = nc.dram_tensor("cbf", [1, 1], f32, kind="ExternalInput")
    d_vw0 = nc.dram_tensor("vw0", [VOL_IN, W], f32, kind="ExternalInput")
    d_vb0 = nc.dram_tensor("vb0", [1, W], f32, kind="ExternalInput")
    d_vwb = nc.dram_tensor("vwb", [DEPTH, W, W], f32, kind="ExternalInput")
    d_vbb = nc.dram_tensor("vbb", [DEPTH, W], f32, kind="ExternalInput")
    d_vwf = nc.dram_tensor("vwf", [W, 1], f32, kind="ExternalInput")
    d_vbf = nc.dram_tensor("vbf", [1, 1], f32, kind="ExternalInput")

    d_pcap = nc.dram_tensor("pcap", [NSH, V], f32, kind="ExternalOutput")
    d_pvol = nc.dram_tensor("pvol", [NSH, 1], f32, kind="ExternalOutput")
    d_mean = nc.dram_tensor("meano", [NSH, W], f32, kind="ExternalOutput")
    d_lsig = nc.dram_tensor("lsigo", [NSH, W], f32, kind="ExternalOutput")
    d_cd2f = nc.dram_tensor("cd2f", [MROW, W * W], f32, kind="ExternalOutput")
    d_vd2f = nc.dram_tensor("vd2f", [NSH, W * W], f32, kind="ExternalOutput")

    with TileContext(nc) as tc:
        with (
            tc.tile_pool(name="const", bufs=1) as cp,
            tc.tile_pool(name="work", bufs=2) as wp,
            tc.tile_pool(name="vbuf", bufs=3) as vp,
            tc.tile_pool(name="psum", bufs=4, space="PSUM") as pp,
            tc.tile_pool(name="psmall", bufs=3, space="PSUM") as sp,
        ):
            # ---------- gathers + features ----------
            ind_t = cp.tile([NSH, 1], dt.int32)
            nc.sync.dma_start(out=ind_t[:], in_=d_ind[:, :])
            gath = cp.tile([NSH, 2 * W], f32)
            nc.gpsimd.indirect_dma_start(
                out=gath[:], out_offset=None, in_=d_dict[:, :],
                in_offset=bass.IndirectOffsetOnAxis(ap=ind_t[:, :1], axis=0))
            epsg = cp.tile([NSH, W], f32)
            nc.gpsimd.indirect_dma_start(
                out=epsg[:], out_offset=None, in_=d_eps[:, :],
                in_offset=bass.IndirectOffsetOnAxis(ap=ind_t[:, :1], axis=0))
            nc.sync.dma_start(out=d_mean[:, :], in_=gath[:, 0:W])
            nc.sync.dma_start(out=d_lsig[:, :], in_=gath[:, W:2 * W])

            sig = cp.tile([NSH, W], f32)
            nc.scalar.activation(out=sig[:], in_=gath[:, W:2 * W],
                                 func=Act.Exp, scale=0.5)
            feat = cp.tile([NSH, W], f32)
            nc.vector.tensor_tensor(out=feat[:], in0=sig[:], in1=epsg[:],
                                    op=Alu.mult)
            nc.vector.tensor_tensor(out=feat[:], in0=feat[:], in1=gath[:, 0:W],
                                    op=Alu.add)

            # ---------- per-row scalars ----------
            ctr = cp.tile([NSH, 3], f32)
            nc.sync.dma_start(out=ctr[:], in_=d_centers[:, :])
            mc = cp.tile([NSH, 1], f32)
            nc.sync.dma_start(out=mc[:], in_=d_meas[:, :])
            voltc = cp.tile([V, 1], f32)
            nc.sync.dma_start(out=voltc[:], in_=d_volt[:, :])
            identt = cp.tile([KAUG, KAUG], f32)
            nc.sync.dma_start(out=identt[:], in_=d_ident[:, :])

            evec = cp.tile([NSH, 1], f32)
            nc.scalar.activation(out=evec[:], in_=feat[:, 0:1],
                                 func=Act.Exp, scale=-1.0)
            svec = cp.tile([NSH, 1], f32)
            nc.vector.tensor_scalar_add(out=svec[:], in0=evec[:], scalar1=1e-10)
            csv = cp.tile([NSH, 1], f32)
            nc.vector.tensor_tensor(out=csv[:], in0=ctr[:, 0:1], in1=svec[:],
                                    op=Alu.mult)
            cev = cp.tile([NSH, 1], f32)
            nc.vector.tensor_tensor(out=cev[:], in0=ctr[:, 0:1], in1=evec[:],
                                    op=Alu.mult)
            varv = cp.tile([NSH, 1], f32)
            nc.vector.tensor_tensor(out=varv[:], in0=mc[:], in1=ctr[:, 0:1],
                                    op=Alu.subtract)
            svv = cp.tile([NSH, 1], f32)
            nc.vector.tensor_tensor(out=svv[:], in0=svec[:], in1=varv[:],
                                    op=Alu.mult)

            # ---------- augmented input block XsAug [96, 67] ----------
            xsa = cp.tile([KAUG, CAP_IN], f32)
            nc.gpsimd.memset(xsa[:], 0.0)
            nc.vector.tensor_copy(out=xsa[0:NSH, 0:1], in_=csv[:])
            nc.vector.tensor_copy(out=xsa[0:NSH, 1:3], in_=ctr[:, 1:3])
            nc.vector.tensor_copy(out=xsa[0:NSH, 4:CAP_IN], in_=feat[:, 1:W])
            nc.vector.tensor_copy(out=xsa[NSH:KAUG, 3:4], in_=voltc[:])

            # B^T = W0^T @ XsAug^T: [64, 96].  Column n (<64) is the layer-0
            # contribution of row n's static input; column 64+v is volt[v]*W0[3].
            pxa = sp.tile([CAP_IN, KAUG], f32, tag="sm")
            nc.tensor.transpose(out=pxa[:], in_=xsa[:], identity=identt[:])
            xsaT = cp.tile([CAP_IN, KAUG], f32)
            nc.vector.tensor_copy(out=xsaT[:], in_=pxa[:])
            cw0s = cp.tile([CAP_IN, W], f32)
            nc.sync.dma_start(out=cw0s[:], in_=d_cw0[:, :])
            pbt = sp.tile([W, KAUG], f32, tag="sm")
            nc.tensor.matmul(out=pbt[:], lhsT=cw0s[:], rhs=xsaT[:],
                             start=True, stop=True)
            bT = cp.tile([W, KAUG], f32)
            nc.vector.tensor_copy(out=bT[:], in_=pbt[:])

            # ---------- weights/biases ----------
            def load_net(dw0, db0, dwb, dbb, dwf, dbf, in_dim, pfx):
                b0c = cp.tile([W, 1], f32, tag=pfx + "b0c")
                nc.sync.dma_start(out=b0c[:], in_=db0[0:1, :])
                wfs = cp.tile([W, 1], f32, tag=pfx + "wf")
                nc.sync.dma_start(out=wfs[:], in_=dwf[:, :])
                wf16 = cp.tile([W, 1], f16, tag=pfx + "wf16")
                nc.vector.tensor_copy(out=wf16[:], in_=wfs[:])
                w0c = cp.tile([W, 1], f32, tag=pfx + "w0c")
                nc.sync.dma_start(out=w0c[:], in_=dw0[0:1, :])
                w0c16 = cp.tile([W, 1], f16, tag=pfx + "w0c16")
                nc.vector.tensor_copy(out=w0c16[:], in_=w0c[:])
                bfc = cp.tile([1, 1], f32, tag=pfx + "bf")
                nc.sync.dma_start(out=bfc[:], in_=dbf[:, :])
                wbs, wbT, bbc = [], [], []
                for i in range(DEPTH):
                    wb_i = cp.tile([W, W], f32, tag=f"{pfx}wb{i}")
                    nc.sync.dma_start(out=wb_i[:], in_=dwb[i, :, :])
                    wbs.append(wb_i)
                    pst = sp.tile([W, W], f32, tag="sm")
                    nc.tensor.transpose(out=pst[:], in_=wb_i[:],
                                        identity=identt[0:W, 0:W])
                    wt_i = cp.tile([W, W], f16, tag=f"{pfx}wbT{i}")
                    nc.vector.tensor_copy(out=wt_i[:], in_=pst[:])
                    wbT.append(wt_i)
                    bb_i = cp.tile([W, 1], f32, tag=f"{pfx}bb{i}")
                    nc.sync.dma_start(out=bb_i[:], in_=dbb[i:i + 1, :])
                    bbc.append(bb_i)
                return b0c, wbs, wbT, bbc, wfs, wf16, w0c16, bfc

            cb0c, cwbs, cwbT, cbbc, cwfs, cwf16, cw0c16, cbfc = load_net(
                d_cw0, d_cb0, d_cwb, d_cbb, d_cwf, d_cbf, CAP_IN, "c")
            vb0c, vwbs, vwbT, vbbc, vwfs, vwf16, vw0c16, vbfc = load_net(
                d_vw0, d_vb0, d_vwb, d_vbb, d_vwf, d_vbf, VOL_IN, "v")

            # ---------- cap net: forward + g0 backward, 4 chunks ----------
            yrow = cp.tile([1, MROW], f32)
            grow = cp.tile([1, MROW], f32)
            for k in range(NCH):
                sl = slice(k * CH, (k + 1) * CH)
                # layer-0 pre-activation via broadcast add:
                # h0pre[:, (n,v)] = bT[:, n] + bT[:, 64+v] + b0
                a_b = bT[:, k * NB:(k + 1) * NB].unsqueeze(2).to_broadcast(
                    [W, NB, V])
                c_b = bT[:, NSH:KAUG].unsqueeze(1).to_broadcast([W, NB, V])
                h0pre = wp.tile([W, CH], f32, tag="h0p")
                nc.vector.scalar_tensor_tensor(
                    out=h0pre[:].rearrange("p (a b) -> p a b", b=V),
                    in0=a_b, scalar=cb0c[:, 0:1], in1=c_b,
                    op0=Alu.add, op1=Alu.add)
                h0 = wp.tile([W, CH], f32, tag="h0")
                nc.scalar.activation(out=h0[:], in_=h0pre[:], func=Act.Relu)
                h = [h0]
                for i in range(DEPTH):
                    pi = pp.tile([W, CH], f32, tag="mm")
                    nc.tensor.matmul(out=pi[:], lhsT=cwbs[i][:], rhs=h[-1][:],
                                     start=True, stop=True)
                    hdt = f16 if i == DEPTH - 1 else f32
                    hi = wp.tile([W, CH], hdt, tag=f"h{i + 1}")
                    nc.scalar.activation(out=hi[:], in_=pi[:], func=Act.Relu,
                                         bias=cbbc[i][:], scale=1.0)
                    h.append(hi)
                py = pp.tile([1, CH], f32, tag="mm")
                nc.tensor.matmul(out=py[:], lhsT=cwf16[:], rhs=h[DEPTH][:],
                                 start=True, stop=True)
                nc.vector.tensor_scalar(out=yrow[:, sl], in0=py[:],
                                        scalar1=cbfc[:, 0:1], scalar2=None,
                                        op0=Alu.add)
                # backward for g0 = dy/dx0 (fp16, masks from fp32 h's)
                v = vp.tile([W, CH], f16, tag="v")
                nc.vector.tensor_scalar(out=v[:], in0=h[DEPTH][:], scalar1=0.0,
                                        scalar2=cwfs[:, 0:1], op0=Alu.is_gt,
                                        op1=Alu.mult)
                for i in range(DEPTH - 1, -1, -1):
                    pb = pp.tile([W, CH], f32, tag="mm")
                    nc.tensor.matmul(out=pb[:], lhsT=cwbT[i][:], rhs=v[:],
                                     start=True, stop=True)
                    v2 = vp.tile([W, CH], f16, tag="v")
                    nc.vector.scalar_tensor_tensor(
                        out=v2[:], in0=h[i][:], scalar=0.0, in1=pb[:],
                        op0=Alu.is_gt, op1=Alu.mult)
                    v = v2
                pg = pp.tile([1, CH], f32, tag="mm")
                nc.tensor.matmul(out=pg[:], lhsT=cw0c16[:], rhs=v[:],
                                 start=True, stop=True)
                nc.vector.tensor_copy(out=grow[:, sl], in_=pg[:])

            # ---------- vol net (single 64-wide batch) ----------
            xv = cp.tile([NSH, VOL_IN], f32)
            nc.gpsimd.memset(xv[:], 0.0)
            nc.vector.tensor_copy(out=xv[:, 0:1], in_=csv[:])
            nc.vector.tensor_copy(out=xv[:, 1:3], in_=ctr[:, 1:3])
            nc.vector.tensor_copy(out=xv[:, 3:VOL_IN], in_=feat[:, 1:W])
            pxv = sp.tile([VOL_IN, NSH], f32, tag="sm")
            nc.tensor.transpose(out=pxv[:], in_=xv[:],
                                identity=identt[0:NSH, 0:NSH])
            xvt = cp.tile([VOL_IN, NSH], f32)
            nc.vector.tensor_copy(out=xvt[:], in_=pxv[:])
            vw0s = cp.tile([VOL_IN, W], f32)
            nc.sync.dma_start(out=vw0s[:], in_=d_vw0[:, :])

            p0v = sp.tile([W, NSH], f32, tag="sm")
            nc.tensor.matmul(out=p0v[:], lhsT=vw0s[:], rhs=xvt[:],
                             start=True, stop=True)
            hv = []
            hv0 = cp.tile([W, NSH], f32, tag="hv0")
            nc.scalar.activation(out=hv0[:], in_=p0v[:], func=Act.Relu,
                                 bias=vb0c[:], scale=1.0)
            hv.append(hv0)
            for i in range(DEPTH):
                piv = sp.tile([W, NSH], f32, tag="sm")
                nc.tensor.matmul(out=piv[:], lhsT=vwbs[i][:], rhs=hv[-1][:],
                                 start=True, stop=True)
                hdt = f16 if i == DEPTH - 1 else f32
                hvi = cp.tile([W, NSH], hdt, tag=f"hv{i + 1}")
                nc.scalar.activation(out=hvi[:], in_=piv[:], func=Act.Relu,
                                     bias=vbbc[i][:], scale=1.0)
                hv.append(hvi)
            pyv = sp.tile([1, NSH], f32, tag="sm")
            nc.tensor.matmul(out=pyv[:], lhsT=vwf16[:], rhs=hv[DEPTH][:],
                             start=True, stop=True)
            yvrow = cp.tile([1, NSH], f32)
            nc.vector.tensor_scalar(out=yvrow[:], in0=pyv[:],
                                    scalar1=vbfc[:, 0:1], scalar2=None,
                                    op0=Alu.add)
            uv = cp.tile([W, NSH], f16, tag="uv3")
            nc.vector.tensor_scalar(out=uv[:], in0=hv[DEPTH][:], scalar1=0.0,
                                    scalar2=vwfs[:, 0:1], op0=Alu.is_gt,
                                    op1=Alu.mult)
            for i in range(DEPTH - 1, -1, -1):
                pbv = sp.tile([W, NSH], f32, tag="sm")
                nc.tensor.matmul(out=pbv[:], lhsT=vwbT[i][:], rhs=uv[:],
                                 start=True, stop=True)
                uv2 = cp.tile([W, NSH], f16, tag=f"uv{i}")
                nc.vector.scalar_tensor_tensor(
                    out=uv2[:], in0=hv[i][:], scalar=0.0, in1=pbv[:],
                    op0=Alu.is_gt, op1=Alu.mult)
                uv = uv2
            pgv = sp.tile([1, NSH], f32, tag="sm")
            nc.tensor.matmul(out=pgv[:], lhsT=vw0c16[:], rhs=uv[:],
                             start=True, stop=True)
            gvrow = cp.tile([1, NSH], f32)
            nc.vector.tensor_copy(out=gvrow[:], in_=pgv[:])

            # ---------- epilogue ----------
            y2 = cp.tile([NSH, V], f32)
            nc.sync.dma_start(out=y2[:], in_=yrow[:, :])
            g2 = cp.tile([NSH, V], f32)
            nc.sync.dma_start(out=g2[:], in_=grow[:, :])
            pc_t = cp.tile([NSH, V], f32)
            nc.vector.scalar_tensor_tensor(out=pc_t[:], in0=g2[:],
                                           scalar=svv[:, 0:1], in1=y2[:],
                                           op0=Alu.mult, op1=Alu.add)
            d2_t = cp.tile([NSH, V], f32)
            nc.vector.tensor_scalar_mul(out=d2_t[:], in0=g2[:],
                                        scalar1=cev[:, 0:1])
            nc.sync.dma_start(out=d_pcap[:, :], in_=pc_t[:])
            nc.sync.dma_start(out=d_cd2f[:, 0:1], in_=d2_t[:])

            yv2 = cp.tile([NSH, 1], f32)
            nc.sync.dma_start(out=yv2[:], in_=yvrow[:, :])
            gv2 = cp.tile([NSH, 1], f32)
            nc.sync.dma_start(out=gv2[:], in_=gvrow[:, :])
            pv_t = cp.tile([NSH, 1], f32)
            nc.vector.scalar_tensor_tensor(out=pv_t[:], in0=gv2[:],
                                           scalar=svv[:, 0:1], in1=yv2[:],
                                           op0=Alu.mult, op1=Alu.add)
            d2v_t = cp.tile([NSH, 1], f32)
            nc.vector.tensor_tensor(out=d2v_t[:], in0=gv2[:], in1=cev[:],
                                    op=Alu.mult)
            nc.sync.dma_start(out=d_pvol[:, :], in_=pv_t[:])
            nc.sync.dma_start(out=d_vd2f[:, 0:1], in_=d2v_t[:])

    nc.compile()
    return nc


def _get_program():
    if "nc" not in _CACHE:
        _CACHE["nc"] = _build_program()
    return _CACHE["nc"]


def _make_in_maps(inp):
    f32 = np.float32
    shared = {
        "volt": inp["voltages"].reshape(V, 1).astype(f32),
        "dictk": inp["dict_kernel"].astype(f32),
        "epsk": inp["eps"].astype(f32),
        "ident96": np.eye(KAUG, dtype=f32),
        "cw0": inp["cap_w0"].astype(f32),
        "cb0": inp["cap_b0"].reshape(1, W).astype(f32),
        "cwb": inp["cap_wb"].astype(f32),
        "cbb": inp["cap_bb"].astype(f32),
        "cwf": inp["cap_wf"].reshape(W, 1).astype(f32),
        "cbf": inp["cap_bf"].reshape(1, 1).astype(f32),
        "vw0": inp["vol_w0"].astype(f32),
        "vb0": inp["vol_b0"].reshape(1, W).astype(f32),
        "vwb": inp["vol_wb"].astype(f32),
        "vbb": inp["vol_bb"].astype(f32),
        "vwf": inp["vol_wf"].reshape(W, 1).astype(f32),
        "vbf": inp["vol_bf"].reshape(1, 1).astype(f32),
    }
    ind32 = inp["indecies"].astype(np.int32).reshape(N, 1)
    centers = inp["centers"].astype(f32)
    meas = inp["measured_cycles"].astype(f32).reshape(N, 1)
    in_maps = []
    for c in range(NCORES):
        sl = slice(c * NSH, (c + 1) * NSH)
        m = dict(shared)
        m["centers"] = np.ascontiguousarray(centers[sl])
        m["meas"] = np.ascontiguousarray(meas[sl])
        m["ind"] = np.ascontiguousarray(ind32[sl])
        in_maps.append(m)
    return in_maps


def _assemble(res):
    predicted_cap = np.concatenate([r["pcap"] for r in res], axis=0)
    predicted_vol = np.concatenate([r["pvol"][:, 0] for r in res], axis=0)
    mean = np.concatenate([r["meano"] for r in res], axis=0)
    log_sig = np.concatenate([r["lsigo"] for r in res], axis=0)
    cd2F = np.concatenate([r["cd2f"] for r in res], axis=0).reshape(N * V, W, W)
    vd2F = np.concatenate([r["vd2f"] for r in res], axis=0).reshape(N, W, W)
    return predicted_cap, predicted_vol, mean, log_sig, cd2F, vd2F


def kernel(**inputs):
    from concourse.bass_utils import run_bass_kernel_spmd

    inp = {k: np.ascontiguousarray(np.asarray(v)) for k, v in inputs.items()}
    nc = _get_program()
    in_maps = _make_in_maps(inp)
    res = run_bass_kernel_spmd(nc, in_maps, core_ids=list(range(NCORES)),
                               trace=False).results
    return _assemble(res)
